# revision 1
# baseline (speedup 1.0000x reference)
"""Trainium2 Bass kernel for nn_MultiHeadAttention (B=2, S=2048, d_model=1024, H=16).

Sharding (8 cores): data-parallel over B (2) x tensor-parallel over head groups
(4 groups of 4 heads).  Each core computes its head-group's Q/K/V projections
(column-sharded weights), attention for its 4 heads, and a row-parallel
out_proj partial product.  The host sums the 4 partials per batch (the
"all-reduce") and adds the output bias.

All on-chip layouts are transposed ([feature, seq]) so that:
  - scores are computed directly transposed  S_T[k,q] = Kh @ Qh^T  (no P
    transpose needed before P@V),
  - softmax denominators come from ones-vector matmuls (col-tiled 4-way),
  - the PE array is fully packed for dk=64 heads via row/col tile_position
    pairing (auto-derived from AP base partitions),
  - the k-loop is software-pipelined one stage (scores of tile k overlap
    exp and P@V of tile k-1), inputs stream on both HWDGE queues.

Dtypes: inputs/projections and the P,V operands are fp16 (1 cyc/row on
the PE, fp32 PSUM accumulation everywhere); scores and out_proj operands
are float32r (TF32 path, 1 cyc/row at N>=256; note f32r cannot be
col-tiled -- XBUS budget -- which is why the P@V/sums side is fp16).
"""

import sys
import numpy as np

for _p in ("/opt/trn_rl_repo", "/root/.axon_site/_ro/trn_rl_repo"):
    if _p not in sys.path:
        sys.path.append(_p)

D_MODEL = 1024
NUM_HEADS = 16
DK = 64
B = 2
S = 2048
N_CORES = 8
HPC = 4               # heads per core
E = HPC * DK          # 256 features per core
NQ = 512              # q-chunk size
N_QC = S // NQ        # 4 q chunks
N_KT = S // 128       # 16 k tiles
N_DT = D_MODEL // 128  # 8 contraction tiles for projections

_PROGRAM = None
_RUN_KWARGS = {}      # test harness may set {"trace": True}
_LAST_RESULTS = None  # BassKernelResults of the last run


def _build_program():
    import concourse.bass as bass
    import concourse.mybir as mybir
    from concourse import bacc, tile
    from contextlib import ExitStack

    f32 = mybir.dt.float32
    f32r = mybir.dt.float32r
    bf16 = mybir.dt.bfloat16
    fp16 = mybir.dt.float16
    AF = mybir.ActivationFunctionType

    nc = bacc.Bacc("TRN2", target_bir_lowering=False, debug=False,
                   num_devices=N_CORES)

    # Per-core DRAM I/O (transposed activations, pre-sliced weights)
    qT = nc.dram_tensor("qT", [D_MODEL, S], mybir.dt.float16, kind="ExternalInput").ap()
    kT = nc.dram_tensor("kT", [D_MODEL, S], mybir.dt.float16, kind="ExternalInput").ap()
    vT = nc.dram_tensor("vT", [D_MODEL, S], mybir.dt.float16, kind="ExternalInput").ap()
    wq = nc.dram_tensor("wq", [D_MODEL, E], mybir.dt.float16, kind="ExternalInput").ap()
    wk = nc.dram_tensor("wk", [D_MODEL, E], mybir.dt.float16, kind="ExternalInput").ap()
    wv = nc.dram_tensor("wv", [D_MODEL, E], mybir.dt.float16, kind="ExternalInput").ap()
    wo = nc.dram_tensor("wo", [E, D_MODEL], f32r, kind="ExternalInput").ap()
    bq = nc.dram_tensor("bq", [E, 1], f32, kind="ExternalInput").ap()
    bk = nc.dram_tensor("bk", [E, 1], f32, kind="ExternalInput").ap()
    bv = nc.dram_tensor("bv", [E, 1], f32, kind="ExternalInput").ap()
    onesl = nc.dram_tensor("onesl", [1, 64], f32r, kind="ExternalInput").ap()
    onesk = nc.dram_tensor("onesk", [128, 1], mybir.dt.float16,
                           kind="ExternalInput").ap()
    zT = nc.dram_tensor("zT", [D_MODEL, S], f32, kind="ExternalOutput").ap()

    def r(ap):  # operands are natively f32r now
        return ap

    with tile.TileContext(nc) as tc, ExitStack() as ctx:
        persist = ctx.enter_context(tc.tile_pool(name="persist", bufs=1))
        const = ctx.enter_context(tc.tile_pool(name="const", bufs=1))

        # Weights resident in SBUF: [128, n_dt, E]-style views
        wq_sb = persist.tile([128, N_DT, E], fp16, tag="wq", name="wq")
        wk_sb = persist.tile([128, N_DT, E], fp16, tag="wk", name="wk")
        wv_sb = persist.tile([128, N_DT, E], fp16, tag="wv", name="wv")
        wo_sb = persist.tile([128, 2, D_MODEL], f32r, tag="wo", name="wo")
        # wk/wq first (gate the K/Q projections), split across queues;
        # wv/wo stream later behind the K inputs
        nc.sync.dma_start(wk_sb[:], wk.rearrange("(t p) e -> p t e", p=128))
        nc.scalar.dma_start(wq_sb[:], wq.rearrange("(t p) e -> p t e", p=128))
        nc.scalar.dma_start(wv_sb[:], wv.rearrange("(t p) e -> p t e", p=128))
        nc.sync.dma_start(wo_sb[:], wo.rearrange("(t p) e -> p t e", p=128))

        bq_sb = persist.tile([128, 2], f32, tag="bq", name="bq")
        bk_sb = persist.tile([128, 2], f32, tag="bk", name="bk")
        bv_sb = persist.tile([128, 2], f32, tag="bv", name="bv")
        nc.sync.dma_start(bq_sb[:], bq.rearrange("(m p) o -> p (m o)", p=128))
        nc.sync.dma_start(bk_sb[:], bk.rearrange("(m p) o -> p (m o)", p=128))
        nc.sync.dma_start(bv_sb[:], bv.rearrange("(m p) o -> p (m o)", p=128))

        from concourse.masks import make_identity
        ident = const.tile([128, 128], fp16, tag="ident", name="ident")
        make_identity(nc, ident)
        # host-provided constants: ones column (sums lhsT) and the
        # pair-broadcast selector
        ones_k = const.tile([128, 1], fp16, tag="ones_k", name="ones_k")
        ones_l = const.tile([1, 64], f32r, tag="ones_l", name="ones_l")
        nc.sync.dma_start(ones_k[:], onesk)
        nc.sync.dma_start(ones_l[:], onesl)

        # Projection outputs (transposed): pair tensors hold 2 heads each
        qh = [persist.tile([128, S], f32r, tag=f"qh{p}", name=f"qh{p}") for p in range(2)]
        kh = [persist.tile([128, S], f32r, tag=f"kh{p}", name=f"kh{p}") for p in range(2)]
        # Vh non-transposed [k, e], s-tile-major columns
        vh = persist.tile([128, N_KT * E], fp16, tag="vh", name="vh")
        # normalized attention output (transposed), pair tensors
        ot = [persist.tile([128, S], f32r, tag=f"ot{p}", name=f"ot{p}") for p in range(2)]

        stage_a = ExitStack()
        xpool = stage_a.enter_context(tc.tile_pool(name="xpool", bufs=8))
        apsum = stage_a.enter_context(
            tc.tile_pool(name="apsum", bufs=8, space="PSUM"))

        # vhT: transposed V projection [e, s] (bf16), transposed to vh after
        vhT = [persist.tile([128, S], fp16, tag=f"vhT{m}", name=f"vhT{m}")
               for m in range(2)]

        # ---- Stage A: projections (all transposed orientation) ---------
        dma_engines = (nc.sync, nc.scalar)  # two HWDGE queues
        for which, xdram, w_sb, b_sb, dst in (
            ("k", kT, wk_sb, bk_sb, kh),
            ("q", qT, wq_sb, bq_sb, qh),
            ("v", vT, wv_sb, bv_sb, vhT),
        ):
            # ps[m][n]: out rows m*128, cols n*512
            ps = [[apsum.tile([128, 512], f32, tag="aps", name="aps") for n in range(4)]
                  for m in range(2)]
            for d in range(N_DT):
                xt = xpool.tile([128, S], fp16, tag="xt", name="xt")
                dma_engines[d % 2].dma_start(xt[:], xdram[d * 128:(d + 1) * 128, :])
                for m in range(2):
                    lhsT = w_sb[:, d, m * 128:(m + 1) * 128]
                    for n in range(4):
                        nc.tensor.matmul(
                            ps[m][n][:], r(lhsT), r(xt[:, n * 512:(n + 1) * 512]),
                            start=(d == 0), stop=(d == N_DT - 1))
            for m in range(2):
                for n in range(4):
                    nc.vector.tensor_scalar_add(
                        dst[m][:, n * 512:(n + 1) * 512], ps[m][n][:],
                        b_sb[:, m:m + 1])

        # vh[s, e] = vhT^T via PE transposes (4 blocks per psum bank)
        for st in range(N_KT):
            tp = apsum.tile([128, 512], fp16, tag="aps", name="tps")                 if st % 2 == 0 else tp
            for m in range(2):
                j = (st % 2) * 2 + m
                nc.tensor.matmul(
                    tp[:, j * 128:(j + 1) * 128],
                    vhT[m][:, st * 128:(st + 1) * 128], ident[:],
                    is_transpose=True, start=True, stop=True,
                    skip_group_check=True)
                nc.vector.tensor_copy(
                    vh[:, st * E + m * 128: st * E + (m + 1) * 128],
                    tp[:, j * 128:(j + 1) * 128])

        stage_a.close()

        # ---- Stage B: attention + out_proj, per q-chunk ----------------
        scp = ctx.enter_context(tc.tile_pool(name="scp", bufs=2, space="PSUM"))
        outp = ctx.enter_context(tc.tile_pool(name="outp", bufs=2, space="PSUM"))
        sump = ctx.enter_context(tc.tile_pool(name="sump", bufs=1, space="PSUM"))
        zp = ctx.enter_context(tc.tile_pool(name="zp", bufs=1, space="PSUM"))

        ptp = ctx.enter_context(tc.tile_pool(name="ptp", bufs=8))
        rp = ctx.enter_context(tc.tile_pool(name="rp", bufs=6))
        bcp = ctx.enter_context(tc.tile_pool(name="bcp", bufs=3))
        zsb = ctx.enter_context(tc.tile_pool(name="zsb", bufs=4))

        for qc in range(N_QC):
            q0, q1 = qc * NQ, (qc + 1) * NQ
            outs = [outp.tile([128, NQ], f32, tag="outp", name="outp") for _ in range(2)]
            sums = sump.tile([128, NQ], f32, tag="sums", name="sums")

            def pv_sums(kt, pts):
                # P@V + denominator for k-tile kt (pts = pair pt tiles)
                for p in range(2):
                    for j in range(2):
                        h = 2 * p + j
                        lo, hi = j * 64, (j + 1) * 64
                        ptj = pts[p][:, j * NQ:(j + 1) * NQ]
                        # P@V (col-tiled pair: head j -> out partitions j*64)
                        nc.tensor.matmul(
                            outs[p][lo:hi, :],
                            r(vh[:, kt * E + h * 64: kt * E + (h + 1) * 64]),
                            r(ptj), start=(kt == 0), stop=(kt == N_KT - 1),
                            skip_group_check=True)
                        # softmax denominator (col-tiled 4-way, M=1)
                        nc.tensor.matmul(
                            sums[32 * h:32 * h + 1, :], r(ones_k[:]), r(ptj),
                            start=(kt == 0), stop=(kt == N_KT - 1),
                            tile_position=(0, 32 * h), skip_group_check=True)

            # k-loop software-pipelined one stage deep: scores(kt) issue on
            # PE while exp(kt-1) runs on ACT and pv/sums(kt-1) follows.
            prev_pts = None
            for kt in range(N_KT):
                k0 = kt * 128
                scs = []
                for p in range(2):
                    # both heads' scores side by side in one 2-bank psum tile
                    sc = scp.tile([128, 2 * NQ], f32, tag="sc", name="sc")
                    for j in range(2):
                        lo, hi = j * 64, (j + 1) * 64
                        nc.tensor.matmul(
                            sc[:, j * NQ:(j + 1) * NQ],
                            r(kh[p][lo:hi, k0:k0 + 128]),
                            r(qh[p][lo:hi, q0:q1]), start=True, stop=True)
                    scs.append(sc)
                if prev_pts is not None:
                    pv_sums(kt - 1, prev_pts)
                pts = []
                for p in range(2):
                    # one wide exp per pair (amortizes ACT fixed cost)
                    pt = ptp.tile([128, 2 * NQ], fp16, tag="pt", name="pt")
                    nc.scalar.activation(pt[:], scs[p][:], AF.Exp, scale=0.125)
                    pts.append(pt)
                prev_pts = pts
            pv_sums(N_KT - 1, prev_pts)
            # normalize: ot = outs * (1/sums) broadcast across partitions
            for p in range(2):
                bc_sb = bcp.tile([128, NQ], f32, tag="bc_sb", name="bc_sb")
                for j in range(2):
                    h = 2 * p + j
                    rv = rp.tile([1, NQ], f32r, tag="rv", name="rv")
                    with nc.allow_low_precision(reason="tf32 softmax recip"):
                        nc.vector.reciprocal(rv[:], sums[32 * h:32 * h + 1, :])
                    # rank-1 broadcast of 1/sum across 64 partitions (PE);
                    # separate base-0 psum tile (f32r can't col-tile)
                    bc = scp.tile([64, NQ], f32, tag="sc", name="bcps")
                    nc.tensor.matmul(bc[:], ones_l[:], rv[:],
                                     start=True, stop=True)
                    nc.vector.tensor_copy(bc_sb[j * 64:(j + 1) * 64, :], bc[:])
                nc.vector.tensor_mul(ot[p][:, q0:q1], outs[p][:], bc_sb[:])
            # out_proj partial: zT[e, q-chunk]
            for e in range(8):
                pool_, tag_ = (zp, "zps") if e % 2 == 0 else (sump, "sums")
                zps = pool_.tile([128, NQ], f32, tag=tag_, name="zps")
                for c in range(2):
                    nc.tensor.matmul(
                        zps[:], r(wo_sb[:, c, e * 128:(e + 1) * 128]),
                        r(ot[c][:, q0:q1]), start=(c == 0), stop=(c == 1))
                zt_sb = zsb.tile([128, NQ], f32, tag="zt_sb", name="zt_sb")
                nc.vector.tensor_copy(zt_sb[:], zps[:])
                dma_engines[e % 2].dma_start(
                    zT[e * 128:(e + 1) * 128, q0:q1], zt_sb[:])

    nc.compile()
    return nc


def _get_program():
    global _PROGRAM
    if _PROGRAM is None:
        _PROGRAM = _build_program()
    return _PROGRAM


ONESL_NP = None
ONESK_NP = None


def _init_consts():
    global ONESL_NP, ONESK_NP
    if ONESL_NP is None:
        import ml_dtypes
        ONESL_NP = np.ones((1, 64), dtype=np.float32)
        ONESK_NP = np.ones((128, 1), np.float16)


def _make_in_maps(q, k, v, Wq, bq, Wk, bk, Wv, bv, Wo):
    _init_consts()
    f32 = np.float32
    xT = {}
    for b in range(B):
        xT[("q", b)] = np.ascontiguousarray(q[b].T, dtype=np.float16)
        xT[("k", b)] = np.ascontiguousarray(k[b].T, dtype=np.float16)
        xT[("v", b)] = np.ascontiguousarray(v[b].T, dtype=np.float16)
    wslices = {}
    for g in range(4):
        sl = slice(g * E, (g + 1) * E)
        wslices[("wq", g)] = np.ascontiguousarray(Wq[sl, :].T, dtype=np.float16)
        wslices[("wk", g)] = np.ascontiguousarray(Wk[sl, :].T, dtype=np.float16)
        wslices[("wv", g)] = np.ascontiguousarray(Wv[sl, :].T, dtype=np.float16)
        wslices[("wo", g)] = np.ascontiguousarray(Wo[:, sl].T, dtype=f32)
        wslices[("bq", g)] = np.ascontiguousarray(bq[sl].reshape(E, 1), dtype=f32)
        wslices[("bk", g)] = np.ascontiguousarray(bk[sl].reshape(E, 1), dtype=f32)
        wslices[("bv", g)] = np.ascontiguousarray(bv[sl].reshape(E, 1),
                                                   dtype=f32)
    in_maps = []
    for c in range(N_CORES):
        b, g = c // 4, c % 4
        in_maps.append({
            "onesl": ONESL_NP, "onesk": ONESK_NP,
            "qT": xT[("q", b)], "kT": xT[("k", b)], "vT": xT[("v", b)],
            "wq": wslices[("wq", g)], "wk": wslices[("wk", g)],
            "wv": wslices[("wv", g)], "wo": wslices[("wo", g)],
            "bq": wslices[("bq", g)], "bk": wslices[("bk", g)],
            "bv": wslices[("bv", g)],
        })
    return in_maps


def _numpy_fallback(q, k, v, mask, Wq, bq, Wk, bk, Wv, bv, Wo, bo):
    # Only used if mask is not all-True (never the case for this problem).
    def proj(x, W, b_):
        y = x @ W.T + b_
        return y.reshape(B, S, NUM_HEADS, DK).transpose(0, 2, 1, 3)
    qh, kh, vh = proj(q, Wq, bq), proj(k, Wk, bk), proj(v, Wv, bv)
    sc = np.einsum("bhqd,bhkd->bhqk", qh, kh) / np.sqrt(DK)
    sc = np.where(mask, sc, np.float32(-1e9))
    sc = sc - sc.max(-1, keepdims=True)
    p = np.exp(sc)
    p /= p.sum(-1, keepdims=True)
    o = np.einsum("bhqk,bhkd->bhqd", p, vh)
    o = o.transpose(0, 2, 1, 3).reshape(B, S, D_MODEL)
    return (o @ Wo.T + bo).astype(np.float32)


def kernel(q, k, v, mask, Wq, bq, Wk, bk, Wv, bv, Wo, bo):
    q = np.asarray(q, dtype=np.float32)
    k = np.asarray(k, dtype=np.float32)
    v = np.asarray(v, dtype=np.float32)
    Wq, Wk, Wv, Wo = (np.asarray(w, dtype=np.float32) for w in (Wq, Wk, Wv, Wo))
    bq, bk, bv, bo = (np.asarray(x, dtype=np.float32) for x in (bq, bk, bv, bo))
    if not np.all(np.asarray(mask)):
        return _numpy_fallback(q, k, v, np.asarray(mask), Wq, bq, Wk, bk,
                               Wv, bv, Wo, bo)

    from concourse.bass_utils import run_bass_kernel_spmd
    nc = _get_program()
    in_maps = _make_in_maps(q, k, v, Wq, bq, Wk, bk, Wv, bv, Wo)
    res = run_bass_kernel_spmd(nc, in_maps, core_ids=list(range(N_CORES)),
                               **_RUN_KWARGS)
    global _LAST_RESULTS
    _LAST_RESULTS = res
    out = np.empty((B, S, D_MODEL), dtype=np.float32)
    for b in range(B):
        acc = res.results[4 * b]["zT"].astype(np.float32).copy()
        for g in range(1, 4):
            acc += res.results[4 * b + g]["zT"]
        out[b] = acc.T + bo
    return out



# revision 4
# speedup vs baseline: 1.2508x; 1.2508x over previous
"""Trainium2 Bass kernel for nn_MultiHeadAttention (B=2, S=2048, d_model=1024, H=16).

Sharding (8 cores): data-parallel over B (2) x tensor-parallel over head groups
(4 groups of 4 heads).  Each core computes its head-group's Q/K/V projections
(column-sharded weights), attention for its 4 heads, and a row-parallel
out_proj partial product.  The host sums the 4 partials per batch (the
"all-reduce") and adds the output bias.

v2 design notes (cost-model driven):
  - P@V uses SWAPPED operands: P (exp scores, [k,q] transposed) is the
    stationary operand and V ([k,e]) the moving one, so each matmul's
    moving free dim is 64 (the head dim) instead of 512.  The attention
    output comes out in [q, e] orientation.
  - Softmax denominators ride along as N=1 matmuls (rhs = ones column)
    reusing the loaded P stationary tile: nearly free.
  - Normalization is a per-partition (q) scalar multiply on DVE -- the
    cheap broadcast direction -- followed by PE transposes back to [e, q]
    for the row-parallel out_proj.
  - V projection is computed directly in [s, e] orientation (lhsT = the
    loaded vT tile, moving = Wv^T), so no V transpose stage exists.
  - bk is dropped exactly (softmax shift invariance); bv is folded into
    the host-side output bias (attention weights sum to 1); bq is applied
    on-device (per-partition add during the qh PSUM->SBUF copy).
  - Projection order: V first, then K, then Q (n-chunk 0 first), so the
    attention k-loop (and the ACT exp stream, the critical resource)
    starts as early as possible.
"""

import sys
import numpy as np

for _p in ("/opt/trn_rl_repo", "/root/.axon_site/_ro/trn_rl_repo"):
    if _p not in sys.path:
        sys.path.append(_p)

D_MODEL = 1024
NUM_HEADS = 16
DK = 64
B = 2
S = 2048
N_CORES = 8
HPC = 4               # heads per core
E = HPC * DK          # 256 features per core
NQ = 512              # q-chunk size
N_QC = S // NQ        # 4 q chunks
N_KT = S // 128       # 16 k tiles
N_DT = D_MODEL // 128  # 8 contraction tiles for projections

_PROGRAM = None
_RUN_KWARGS = {}      # test harness may set {"trace": True}
_LAST_RESULTS = None  # BassKernelResults of the last run


def _build_program():
    import concourse.bass as bass
    import concourse.mybir as mybir
    from concourse import bacc, tile
    from contextlib import ExitStack

    f32 = mybir.dt.float32
    fp16 = mybir.dt.float16
    AF = mybir.ActivationFunctionType

    nc = bacc.Bacc("TRN2", target_bir_lowering=False, debug=False,
                   num_devices=N_CORES)

    # Per-core DRAM I/O (transposed activations, pre-sliced weights)
    qT = nc.dram_tensor("qT", [D_MODEL, S], fp16, kind="ExternalInput").ap()
    kT = nc.dram_tensor("kT", [D_MODEL, S], fp16, kind="ExternalInput").ap()
    vT = nc.dram_tensor("vT", [D_MODEL, S], fp16, kind="ExternalInput").ap()
    wq = nc.dram_tensor("wq", [D_MODEL, E], fp16, kind="ExternalInput").ap()
    wk = nc.dram_tensor("wk", [D_MODEL, E], fp16, kind="ExternalInput").ap()
    wv = nc.dram_tensor("wv", [D_MODEL, E], fp16, kind="ExternalInput").ap()
    wo = nc.dram_tensor("wo", [E, D_MODEL], fp16, kind="ExternalInput").ap()
    bq = nc.dram_tensor("bq", [E, 1], f32, kind="ExternalInput").ap()
    onesk = nc.dram_tensor("onesk", [128, 1], fp16, kind="ExternalInput").ap()
    zT = nc.dram_tensor("zT", [D_MODEL, S], fp16, kind="ExternalOutput").ap()

    with tile.TileContext(nc) as tc, ExitStack() as ctx:
        persist = ctx.enter_context(tc.tile_pool(name="persist", bufs=1))
        const = ctx.enter_context(tc.tile_pool(name="const", bufs=1))

        # Weights resident in SBUF
        wv_sb = persist.tile([128, N_DT, E], fp16, tag="wv", name="wv")
        wk_sb = persist.tile([128, N_DT, E], fp16, tag="wk", name="wk")
        wq_sb = persist.tile([128, N_DT, E], fp16, tag="wq", name="wq")
        wo_sb = persist.tile([128, 2, D_MODEL], fp16, tag="wo", name="wo")
        bq_sb = persist.tile([128, 2], f32, tag="bq", name="bq")

        from concourse.masks import make_identity
        ident = const.tile([128, 128], fp16, tag="ident", name="ident")
        make_identity(nc, ident)
        ones_k = const.tile([128, 1], fp16, tag="ones_k", name="ones_k")

        # Projection outputs: qh/kh transposed pair tensors ([2-head x 64, S]);
        # vh NON-transposed [s, e] (s-tile-major along dim1)
        qh = [persist.tile([128, S], fp16, tag=f"qh{p}", name=f"qh{p}")
              for p in range(2)]
        kh = [persist.tile([128, S], fp16, tag=f"kh{p}", name=f"kh{p}")
              for p in range(2)]
        vh = persist.tile([128, N_KT, E], fp16, tag="vh", name="vh")
        # attention output transposed back to [e, q] for out_proj
        ot = [persist.tile([128, S], fp16, tag=f"ot{p}", name=f"ot{p}")
              for p in range(2)]
        # out_proj partials staged for one big DMA per e-tile
        zsb = [persist.tile([128, S], fp16, tag=f"zsb{e}", name=f"zsb{e}")
               for e in range(N_DT)]

        # ---- DMA schedule: weights + x tiles, dual queue ----------------
        nc.sync.dma_start(wv_sb[:], wv.rearrange("(t p) e -> p t e", p=128))
        xpool = ctx.enter_context(tc.tile_pool(name="xpool", bufs=16))
        xv = [xpool.tile([128, S], fp16, tag="xt", name=f"xv{d}")
              for d in range(N_DT)]
        for d in range(N_DT):
            eng = nc.sync if d % 2 == 0 else nc.scalar
            eng.dma_start(xv[d][:], vT[d * 128:(d + 1) * 128, :])
        nc.sync.dma_start(wk_sb[:], wk.rearrange("(t p) e -> p t e", p=128))
        xk = [xpool.tile([128, S], fp16, tag="xt", name=f"xk{d}")
              for d in range(N_DT)]
        for d in range(N_DT):
            eng = nc.sync if d % 2 == 0 else nc.scalar
            eng.dma_start(xk[d][:], kT[d * 128:(d + 1) * 128, :])
        nc.sync.dma_start(wq_sb[:], wq.rearrange("(t p) e -> p t e", p=128))
        xq = [xpool.tile([128, S], fp16, tag="xt", name=f"xq{d}")
              for d in range(N_DT)]
        for d in range(N_DT):
            eng = nc.sync if d % 2 == 0 else nc.scalar
            eng.dma_start(xq[d][:], qT[d * 128:(d + 1) * 128, :])
        nc.sync.dma_start(wo_sb[:], wo.rearrange("(t p) e -> p t e", p=128))
        nc.sync.dma_start(bq_sb[:], bq.rearrange("(m p) o -> p (m o)", p=128))
        nc.sync.dma_start(ones_k[:], onesk)

        # ---- Stage A: projections ---------------------------------------
        stage_a = ExitStack()
        apsum = stage_a.enter_context(
            tc.tile_pool(name="apsum", bufs=4, space="PSUM"))

        # V projection directly in [s, e] orientation:
        #   vh[s, e] = sum_d vT[d, s] * WvT[d, e]
        for st in range(N_KT):
            vps = apsum.tile([128, E], f32, tag="vps", name="vps")
            for d in range(N_DT):
                nc.tensor.matmul(
                    vps[:], xv[d][:, st * 128:(st + 1) * 128], wv_sb[:, d, :],
                    start=(d == 0), stop=(d == N_DT - 1))
            nc.vector.tensor_copy(vh[:, st, :], vps[:])

        # K projection (transposed, no bias -- exact by softmax invariance),
        # then Q (transposed, +bq).  n-chunk 0 first for both so the first
        # q-chunk's attention can start while n1-3 still project.
        def proj_chunk(xs, w_sb, dst, n, m, bias):
            ps = apsum.tile([128, NQ], f32, tag="kqps", name="kqps")
            for d in range(N_DT):
                nc.tensor.matmul(
                    ps[:], w_sb[:, d, m * 128:(m + 1) * 128],
                    xs[d][:, n * NQ:(n + 1) * NQ],
                    start=(d == 0), stop=(d == N_DT - 1))
            if bias is None:
                nc.vector.tensor_copy(dst[m][:, n * NQ:(n + 1) * NQ], ps[:])
            else:
                nc.vector.tensor_scalar_add(
                    dst[m][:, n * NQ:(n + 1) * NQ], ps[:], bias[:, m:m + 1])

        for m in range(2):
            proj_chunk(xk, wk_sb, kh, 0, m, None)
        for m in range(2):
            proj_chunk(xq, wq_sb, qh, 0, m, bq_sb)
        for n in range(1, 4):
            for m in range(2):
                proj_chunk(xk, wk_sb, kh, n, m, None)
            for m in range(2):
                proj_chunk(xq, wq_sb, qh, n, m, bq_sb)

        stage_a.close()

        # ---- Stage B: attention + out_proj, per q-chunk -----------------
        scp = ctx.enter_context(tc.tile_pool(name="scp", bufs=2, space="PSUM"))
        outp = ctx.enter_context(tc.tile_pool(name="outp", bufs=1, space="PSUM"))
        sump = ctx.enter_context(tc.tile_pool(name="sump", bufs=1, space="PSUM"))
        drp = ctx.enter_context(tc.tile_pool(name="drp", bufs=1, space="PSUM"))

        ptp = ctx.enter_context(tc.tile_pool(name="ptp", bufs=12))
        rp = ctx.enter_context(tc.tile_pool(name="rp", bufs=2))
        bcp = ctx.enter_context(tc.tile_pool(name="bcp", bufs=8))

        for qc in range(N_QC):
            q0, q1 = qc * NQ, (qc + 1) * NQ
            # [q, e] attention accumulator: 4 q-subtiles x (4 heads x 64)
            out2 = outp.tile([128, 4, E], f32, tag="out2", name="out2")
            sums = sump.tile([128, 16], f32, tag="sums", name="sums")

            def pv_sums(kt, pts):
                # PSUM start_tensor_calc zeroes a whole 2KB bank region, so
                # only the FIRST matmul touching each bank may set start=True
                # (out2 spans 2 banks: qt0/qt1 and qt2/qt3; sums is 1 bank).
                for p in range(2):
                    for j in range(2):
                        h = 2 * p + j
                        for qt in range(4):
                            first = p == 0 and j == 0
                            lhsT = pts[p][:, j * NQ + qt * 128:
                                          j * NQ + (qt + 1) * 128]
                            nc.tensor.matmul(
                                out2[:, qt, h * 64:(h + 1) * 64], lhsT,
                                vh[:, kt, h * 64:(h + 1) * 64],
                                start=(kt == 0 and first and qt % 2 == 0),
                                stop=(kt == N_KT - 1),
                                skip_group_check=True)
                            nc.tensor.matmul(
                                sums[:, qt * 4 + h:qt * 4 + h + 1], lhsT,
                                ones_k[:],
                                start=(kt == 0 and first and qt == 0),
                                stop=(kt == N_KT - 1),
                                skip_group_check=True)

            # k-loop software-pipelined one stage deep
            prev_pts = None
            for kt in range(N_KT):
                k0 = kt * 128
                scs = []
                for p in range(2):
                    sc = scp.tile([128, 2 * NQ], f32, tag="sc", name="sc")
                    for j in range(2):
                        lo, hi = j * 64, (j + 1) * 64
                        nc.tensor.matmul(
                            sc[:, j * NQ:(j + 1) * NQ],
                            kh[p][lo:hi, k0:k0 + 128],
                            qh[p][lo:hi, q0:q1], start=True, stop=True)
                    scs.append(sc)
                if prev_pts is not None:
                    pv_sums(kt - 1, prev_pts)
                pts = []
                for p in range(2):
                    pt = ptp.tile([128, 2 * NQ], fp16, tag="pt", name="pt")
                    nc.scalar.activation(pt[:], scs[p][:], AF.Exp, scale=0.125)
                    pts.append(pt)
                prev_pts = pts
            pv_sums(N_KT - 1, prev_pts)

            # drain: normalize in [q, e], transpose to [e, q], out_proj
            rv = rp.tile([128, 16], f32, tag="rv", name="rv")
            nc.vector.reciprocal(rv[:], sums[:])
            o2n = []
            for qt in range(4):
                o2 = bcp.tile([128, E], fp16, tag="o2n", name="o2n")
                for h in range(4):
                    nc.vector.tensor_scalar_mul(
                        o2[:, h * 64:(h + 1) * 64],
                        out2[:, qt, h * 64:(h + 1) * 64],
                        rv[:, qt * 4 + h:qt * 4 + h + 1])
                o2n.append(o2)
            tp = drp.tile([128, 1024], fp16, tag="scr", name="tp")
            for qt in range(4):
                for et in range(2):
                    blk = qt * 2 + et
                    nc.tensor.matmul(
                        tp[:, blk * 128:(blk + 1) * 128],
                        o2n[qt][:, et * 128:(et + 1) * 128], ident[:],
                        is_transpose=True, start=True, stop=True,
                        skip_group_check=True)
                    nc.vector.tensor_copy(
                        ot[et][:, q0 + qt * 128:q0 + (qt + 1) * 128],
                        tp[:, blk * 128:(blk + 1) * 128])
            # out_proj partial: zT[e, q-chunk] staged into zsb
            for e in range(N_DT):
                zps = drp.tile([128, NQ], f32, tag="scr", name="zps")
                for c in range(2):
                    nc.tensor.matmul(
                        zps[:], wo_sb[:, c, e * 128:(e + 1) * 128],
                        ot[c][:, q0:q1], start=(c == 0), stop=(c == 1))
                nc.vector.tensor_copy(zsb[e][:, q0:q1], zps[:])

        for e in range(N_DT):
            nc.sync.dma_start(zT[e * 128:(e + 1) * 128, :], zsb[e][:])

    nc.compile()
    return nc


def _get_program():
    global _PROGRAM
    if _PROGRAM is None:
        _PROGRAM = _build_program()
    return _PROGRAM


ONESK_NP = None


def _init_consts():
    global ONESK_NP
    if ONESK_NP is None:
        ONESK_NP = np.ones((128, 1), np.float16)


def _make_in_maps(q, k, v, Wq, bq, Wk, Wv, Wo):
    _init_consts()
    f32 = np.float32
    xT = {}
    for b in range(B):
        xT[("q", b)] = np.ascontiguousarray(q[b].T, dtype=np.float16)
        xT[("k", b)] = np.ascontiguousarray(k[b].T, dtype=np.float16)
        xT[("v", b)] = np.ascontiguousarray(v[b].T, dtype=np.float16)
    wslices = {}
    for g in range(4):
        sl = slice(g * E, (g + 1) * E)
        wslices[("wq", g)] = np.ascontiguousarray(Wq[sl, :].T, dtype=np.float16)
        wslices[("wk", g)] = np.ascontiguousarray(Wk[sl, :].T, dtype=np.float16)
        wslices[("wv", g)] = np.ascontiguousarray(Wv[sl, :].T, dtype=np.float16)
        wslices[("wo", g)] = np.ascontiguousarray(Wo[:, sl].T, dtype=np.float16)
        wslices[("bq", g)] = np.ascontiguousarray(
            bq[sl].reshape(E, 1), dtype=f32)
    in_maps = []
    for c in range(N_CORES):
        b, g = c // 4, c % 4
        in_maps.append({
            "onesk": ONESK_NP,
            "qT": xT[("q", b)], "kT": xT[("k", b)], "vT": xT[("v", b)],
            "wq": wslices[("wq", g)], "wk": wslices[("wk", g)],
            "wv": wslices[("wv", g)], "wo": wslices[("wo", g)],
            "bq": wslices[("bq", g)],
        })
    return in_maps


def _numpy_fallback(q, k, v, mask, Wq, bq, Wk, bk, Wv, bv, Wo, bo):
    # Only used if mask is not all-True (never the case for this problem).
    def proj(x, W, b_):
        y = x @ W.T + b_
        return y.reshape(B, S, NUM_HEADS, DK).transpose(0, 2, 1, 3)
    qh, kh, vh = proj(q, Wq, bq), proj(k, Wk, bk), proj(v, Wv, bv)
    sc = np.einsum("bhqd,bhkd->bhqk", qh, kh) / np.sqrt(DK)
    sc = np.where(mask, sc, np.float32(-1e9))
    sc = sc - sc.max(-1, keepdims=True)
    p = np.exp(sc)
    p /= p.sum(-1, keepdims=True)
    o = np.einsum("bhqk,bhkd->bhqd", p, vh)
    o = o.transpose(0, 2, 1, 3).reshape(B, S, D_MODEL)
    return (o @ Wo.T + bo).astype(np.float32)


def kernel(q, k, v, mask, Wq, bq, Wk, bk, Wv, bv, Wo, bo):
    q = np.asarray(q, dtype=np.float32)
    k = np.asarray(k, dtype=np.float32)
    v = np.asarray(v, dtype=np.float32)
    Wq, Wk, Wv, Wo = (np.asarray(w, dtype=np.float32) for w in (Wq, Wk, Wv, Wo))
    bq, bk, bv, bo = (np.asarray(x, dtype=np.float32) for x in (bq, bk, bv, bo))
    if not np.all(np.asarray(mask)):
        return _numpy_fallback(q, k, v, np.asarray(mask), Wq, bq, Wk, bk,
                               Wv, bv, Wo, bo)

    from concourse.bass_utils import run_bass_kernel_spmd
    nc = _get_program()
    in_maps = _make_in_maps(q, k, v, Wq, bq, Wk, Wv, Wo)
    res = run_bass_kernel_spmd(nc, in_maps, core_ids=list(range(N_CORES)),
                               **_RUN_KWARGS)
    global _LAST_RESULTS
    _LAST_RESULTS = res
    # bk is dropped on-device (exact: softmax shift invariance); bv is
    # folded into the output bias (attention weights sum to 1).
    bo_eff = bo + Wo @ bv
    out = np.empty((B, S, D_MODEL), dtype=np.float32)
    for b in range(B):
        acc = res.results[4 * b]["zT"].astype(np.float32)
        for g in range(1, 4):
            acc = acc + res.results[4 * b + g]["zT"].astype(np.float32)
        out[b] = acc.T + bo_eff
    return out


# revision 10
# speedup vs baseline: 1.4824x; 1.1851x over previous
"""Trainium2 Bass kernel for nn_MultiHeadAttention (B=2, S=2048, d_model=1024, H=16).

Sharding (8 cores): data-parallel over B (2) x tensor-parallel over head groups
(4 groups of 4 heads).  Each core computes its head-group's Q/K/V projections
(column-sharded weights), attention for its 4 heads, and a row-parallel
out_proj partial product.  The host sums the 4 partials per batch (the
"all-reduce") and adds the output bias.

v3 design notes (cost-model driven):
  - P@V uses SWAPPED operands: P (exp scores, [k,q]) stationary, V ([k,e])
    moving, so the moving free dim is 64 instead of 512; attention output
    lands in [q, e].  Softmax denominators ride along as N=1 matmuls
    (rhs = ones) reusing the loaded P stationary tile.
  - Normalization = per-partition scalar multiply on DVE; PE transposes
    bring [q, e] back to [e, q] for the row-parallel out_proj.
  - PSUM is a single 8-bank working set shared by EVERYTHING (no stacked
    stage pools, which would serialize projections before attention):
    sc 2x2 banks, out2 2, sums 1, scratch 1.  Projections beyond the
    first k/q n-chunk are drip-fed through the scratch bank inside the
    attention loop (deadline-ordered backlog), so the ACT exp stream --
    the critical resource -- starts ~15us in instead of ~65us.
  - x is loaded in [128, d, 512] n-chunks (one DMA each) so the first
    chunk of K and Q arrives after ~9us of serial DMA instead of ~30us.
  - bk is dropped exactly (softmax shift invariance); bv is folded into
    the host-side output bias (attention weights sum to 1); bq is applied
    on-device during the qh PSUM->SBUF copy.
  - PSUM start_tensor_calc zeroing is bank-granular: only the first
    matmul touching a bank in an accumulation group sets start=True.
"""

import sys
import numpy as np

for _p in ("/opt/trn_rl_repo", "/root/.axon_site/_ro/trn_rl_repo"):
    if _p not in sys.path:
        sys.path.append(_p)

D_MODEL = 1024
NUM_HEADS = 16
DK = 64
B = 2
S = 2048
N_CORES = 8
HPC = 4               # heads per core
E = HPC * DK          # 256 features per core
NQ = 512              # q-chunk size
N_QC = S // NQ        # 4 q chunks
N_KT = S // 128       # 16 k tiles
N_DT = D_MODEL // 128  # 8 contraction tiles for projections

_PROGRAM = None
_RUN_KWARGS = {}      # test harness may set {"trace": True}
_LAST_RESULTS = None  # BassKernelResults of the last run

# Backlog draw schedule: how many deferred projection chunks to emit
# after each (qc, kt) iteration of the attention loop.
_DRAW = {0: [1, 1, 1, 1, 1, 1, 1, 1, 1, 2, 1, 2, 2, 2, 1, 1],
         1: [1, 1, 1, 1] + [0] * 12}


def _build_program():
    import concourse.bass as bass
    import concourse.mybir as mybir
    from concourse import bacc, tile
    from contextlib import ExitStack

    f32 = mybir.dt.float32
    fp16 = mybir.dt.float16
    i16 = mybir.dt.int16
    AF = mybir.ActivationFunctionType
    ALU = mybir.AluOpType
    # Schraudolph fast-exp constants (int16/fp16 bitcast):
    #   i16 = round(s * 0.125 * 1024/ln2 + (15*1024 - C));  C tuned for
    #   min RMS rel error (~1.8%); applied to ~22% of exp tiles on DVE.
    SCH_A = 0.125 * 1024.0 / np.log(2.0)
    SCH_B = 15.0 * 1024.0 - 60.0

    nc = bacc.Bacc("TRN2", target_bir_lowering=False, debug=False,
                   num_devices=N_CORES)

    qT = nc.dram_tensor("qT", [D_MODEL, S], fp16, kind="ExternalInput").ap()
    kT = nc.dram_tensor("kT", [D_MODEL, S], fp16, kind="ExternalInput").ap()
    vT = nc.dram_tensor("vT", [D_MODEL, S], fp16, kind="ExternalInput").ap()
    wq = nc.dram_tensor("wq", [D_MODEL, E], fp16, kind="ExternalInput").ap()
    wk = nc.dram_tensor("wk", [D_MODEL, E], fp16, kind="ExternalInput").ap()
    wv = nc.dram_tensor("wv", [D_MODEL, E], fp16, kind="ExternalInput").ap()
    wo = nc.dram_tensor("wo", [E, D_MODEL], fp16, kind="ExternalInput").ap()
    bq = nc.dram_tensor("bq", [E, 1], f32, kind="ExternalInput").ap()
    onesk = nc.dram_tensor("onesk", [128, 1], fp16, kind="ExternalInput").ap()
    zT = nc.dram_tensor("zT", [D_MODEL, S], fp16, kind="ExternalOutput").ap()

    with tile.TileContext(nc) as tc, ExitStack() as ctx:
        persist = ctx.enter_context(tc.tile_pool(name="persist", bufs=1))
        const = ctx.enter_context(tc.tile_pool(name="const", bufs=1))

        wv_sb = persist.tile([128, N_DT, E], fp16, tag="wv", name="wv")
        wk_sb = persist.tile([128, N_DT, E], fp16, tag="wk", name="wk")
        wq_sb = persist.tile([128, N_DT, E], fp16, tag="wq", name="wq")
        wo_sb = persist.tile([128, 2, D_MODEL], fp16, tag="wo", name="wo")
        bq_sb = persist.tile([128, 2], f32, tag="bq", name="bq")

        from concourse.masks import make_identity
        ident = const.tile([128, 128], fp16, tag="ident", name="ident")
        make_identity(nc, ident)
        ones_k = const.tile([128, 1], fp16, tag="ones_k", name="ones_k")

        qh = [persist.tile([128, S], fp16, tag=f"qh{p}", name=f"qh{p}")
              for p in range(2)]
        kh = [persist.tile([128, S], fp16, tag=f"kh{p}", name=f"kh{p}")
              for p in range(2)]
        vh = persist.tile([128, N_KT, E], fp16, tag="vh", name="vh")
        ot = [persist.tile([128, S], fp16, tag=f"ot{p}", name=f"ot{p}")
              for p in range(2)]

        # ---- x chunk tiles + DMA schedule (priority order) --------------
        xpool = ctx.enter_context(tc.tile_pool(name="xpool", bufs=12))
        xk = [xpool.tile([128, N_DT, NQ], fp16, tag="xt", name=f"xk{n}")
              for n in range(4)]
        xq = [xpool.tile([128, N_DT, NQ], fp16, tag="xt", name=f"xq{n}")
              for n in range(4)]
        xv = [xpool.tile([128, N_DT, NQ], fp16, tag="xt", name=f"xv{n}")
              for n in range(4)]
        kT3 = kT.rearrange("(t p) s -> p t s", p=128)
        qT3 = qT.rearrange("(t p) s -> p t s", p=128)
        vT3 = vT.rearrange("(t p) s -> p t s", p=128)

        nc.sync.dma_start(wv_sb[:], wv.rearrange("(t p) e -> p t e", p=128))
        nc.scalar.dma_start(ones_k[:], onesk)
        nc.sync.dma_start(xv[0][:], vT3[:, :, 0:NQ])
        nc.scalar.dma_start(wk_sb[:], wk.rearrange("(t p) e -> p t e", p=128))
        nc.sync.dma_start(xk[0][:], kT3[:, :, 0:NQ])
        nc.scalar.dma_start(wq_sb[:], wq.rearrange("(t p) e -> p t e", p=128))
        nc.sync.dma_start(xq[0][:], qT3[:, :, 0:NQ])
        nc.scalar.dma_start(bq_sb[:], bq.rearrange("(m p) o -> p (m o)", p=128))
        for n in range(1, 4):
            nc.sync.dma_start(xv[n][:], vT3[:, :, n * NQ:(n + 1) * NQ])
            nc.scalar.dma_start(xk[n][:], kT3[:, :, n * NQ:(n + 1) * NQ])
            nc.sync.dma_start(xq[n][:], qT3[:, :, n * NQ:(n + 1) * NQ])
        nc.scalar.dma_start(wo_sb[:], wo.rearrange("(t p) e -> p t e", p=128))

        # ---- PSUM pools: one shared 8-bank working set ------------------
        scp = ctx.enter_context(tc.tile_pool(name="scp", bufs=4, space="PSUM"))
        outp = ctx.enter_context(tc.tile_pool(name="outp", bufs=1, space="PSUM"))
        sump = ctx.enter_context(tc.tile_pool(name="sump", bufs=1, space="PSUM"))
        scr = ctx.enter_context(tc.tile_pool(name="scr", bufs=1, space="PSUM"))

        ptp = ctx.enter_context(tc.tile_pool(name="ptp", bufs=28))
        rp = ctx.enter_context(tc.tile_pool(name="rp", bufs=2))
        bcp = ctx.enter_context(tc.tile_pool(name="bcp", bufs=8))
        zsbp = ctx.enter_context(tc.tile_pool(name="zsbp", bufs=4))

        # ---- first K/Q n-chunk on the (still idle) score slots ----------
        def proj_big(xs, w_sb, dst, n, bias):
            for m in range(2):
                ps = scp.tile([128, NQ], f32, tag="sc", name="projbig")
                for d in range(N_DT):
                    nc.tensor.matmul(
                        ps[:], w_sb[:, d, m * 128:(m + 1) * 128],
                        xs[n][:, d, :],
                        start=(d == 0), stop=(d == N_DT - 1))
                if bias is None:
                    nc.vector.tensor_copy(
                        dst[m][:, n * NQ:(n + 1) * NQ], ps[:])
                else:
                    nc.vector.tensor_scalar_add(
                        dst[m][:, n * NQ:(n + 1) * NQ], ps[:],
                        bias[:, m:m + 1])

        # V0-3 run during the kT/qT DMA wait and warm up the PE p-state
        # (they only need wv + the first vT chunk, which load first).
        _V_PRE = 4

        # ---- deferred projection backlog (drip-fed through scratch) -----
        def emit_v(st):
            vps = scr.tile([128, E], f32, tag="scr", name="vps")
            n, col = st // 4, (st % 4) * 128
            for d in range(N_DT):
                nc.tensor.matmul(
                    vps[:], xv[n][:, d, col:col + 128], wv_sb[:, d, :],
                    start=(d == 0), stop=(d == N_DT - 1))
            nc.vector.tensor_copy(vh[:, st, :], vps[:])

        def emit_kq_chunk(xs, w_sb, dst, n, m, bias):
            ps = scr.tile([128, NQ], f32, tag="scr", name="kqps")
            for d in range(N_DT):
                nc.tensor.matmul(
                    ps[:], w_sb[:, d, m * 128:(m + 1) * 128], xs[n][:, d, :],
                    start=(d == 0), stop=(d == N_DT - 1))
            if bias is None:
                nc.vector.tensor_copy(dst[m][:, n * NQ:(n + 1) * NQ], ps[:])
            else:
                nc.vector.tensor_scalar_add(
                    dst[m][:, n * NQ:(n + 1) * NQ], ps[:], bias[:, m:m + 1])

        for st in range(_V_PRE):
            emit_v(st)
        proj_big(xk, wk_sb, kh, 0, None)
        proj_big(xq, wq_sb, qh, 0, bq_sb)

        backlog = []
        _K = lambda n, m: (lambda: emit_kq_chunk(xk, wk_sb, kh, n, m, None))
        _Q = lambda n, m: (lambda: emit_kq_chunk(xq, wq_sb, qh, n, m, bq_sb))
        _V = lambda st: (lambda: emit_v(st))
        backlog += [_V(4), _K(1, 0), _V(5), _K(1, 1), _V(6), _K(2, 0),
                    _V(7), _K(2, 1), _V(8), _V(9), _K(3, 0), _V(10),
                    _V(11), _K(3, 1), _V(12), _Q(1, 0), _V(13), _Q(1, 1),
                    _V(14), _V(15), _Q(2, 0), _Q(2, 1), _Q(3, 0), _Q(3, 1)]
        backlog = backlog[::-1]  # pop() from the front

        # ---- attention + out_proj, per q-chunk --------------------------
        for qc in range(N_QC):
            q0, q1 = qc * NQ, (qc + 1) * NQ
            out2 = outp.tile([128, 4, E], f32, tag="out2", name="out2")
            sums = sump.tile([128, 16], f32, tag="sums", name="sums")

            def pv_sums(kt, pts):
                # only the FIRST matmul touching each PSUM bank of an
                # accumulation group may set start=True (bank-granular zero)
                for h in range(4):
                    for qt in range(4):
                        lhsT = pts[h][:, qt * 128:(qt + 1) * 128]
                        nc.tensor.matmul(
                            out2[:, qt, h * 64:(h + 1) * 64], lhsT,
                            vh[:, kt, h * 64:(h + 1) * 64],
                            start=(kt == 0 and h == 0 and qt % 2 == 0),
                            stop=(kt == N_KT - 1),
                            skip_group_check=True)
                        nc.tensor.matmul(
                            sums[:, qt * 4 + h:qt * 4 + h + 1], lhsT,
                            ones_k[:],
                            start=(kt == 0 and h == 0 and qt == 0),
                            stop=(kt == N_KT - 1),
                            skip_group_check=True)

            draw = _DRAW.get(qc, [0] * N_KT)
            prev_pts = None
            for kt in range(N_KT):
                k0 = kt * 128
                scs = []
                for h in range(4):
                    p, j = h // 2, h % 2
                    lo, hi = j * 64, (j + 1) * 64
                    sc = scp.tile([128, NQ], f32, tag="sc", name="sc")
                    nc.tensor.matmul(
                        sc[:], kh[p][lo:hi, k0:k0 + 128],
                        qh[p][lo:hi, q0:q1], start=True, stop=True)
                    scs.append(sc)
                if prev_pts is not None:
                    pv_sums(kt - 1, prev_pts)
                for _ in range(draw[kt]):
                    if backlog:
                        backlog.pop()()
                pts = []
                for h in range(4):
                    off = ((kt * 4 + h) % 16 == 9 if qc == 0
                           else (kt * 4 + h) % 8 in (1, 4, 6))
                    if off:
                        pti = ptp.tile([128, NQ], i16, tag="pt", name="pti")
                        nc.vector.tensor_scalar(
                            pti[:], scs[h][:], SCH_A, SCH_B,
                            ALU.mult, ALU.add)
                        pt = pti.bitcast(fp16)
                    else:
                        pt = ptp.tile([128, NQ], fp16, tag="pt", name="pt")
                        nc.scalar.activation(pt[:], scs[h][:], AF.Exp,
                                             scale=0.125)
                    pts.append(pt)
                prev_pts = pts
            pv_sums(N_KT - 1, prev_pts)

            # drain: normalize in [q, e], transpose to [e, q], out_proj.
            # For the last q-chunk (nothing left to hide behind) the drain
            # is latency-critical: split the normalize between DVE and the
            # now-idle ACT, and pipeline out_proj on the freed score slots.
            last = qc == N_QC - 1
            rv = rp.tile([128, 16], f32, tag="rv", name="rv")
            nc.vector.reciprocal(rv[:], sums[:])
            o2n = []
            for qt in range(4):
                o2 = bcp.tile([128, E], fp16, tag="o2n", name="o2n")
                for h in range(4):
                    c0 = qt * 4 + h
                    if last and h % 2 == 1:
                        nc.scalar.activation(
                            o2[:, h * 64:(h + 1) * 64],
                            out2[:, qt, h * 64:(h + 1) * 64],
                            AF.Copy, scale=rv[:, c0:c0 + 1])
                    else:
                        nc.vector.tensor_scalar_mul(
                            o2[:, h * 64:(h + 1) * 64],
                            out2[:, qt, h * 64:(h + 1) * 64],
                            rv[:, c0:c0 + 1])
                o2n.append(o2)
            tp = scr.tile([128, 1024], fp16, tag="scr", name="tp")
            for qt in range(4):
                for et in range(2):
                    blk = qt * 2 + et
                    nc.tensor.matmul(
                        tp[:, blk * 128:(blk + 1) * 128],
                        o2n[qt][:, et * 128:(et + 1) * 128], ident[:],
                        is_transpose=True, start=True, stop=True,
                        skip_group_check=True)
                    nc.vector.tensor_copy(
                        ot[et][:, q0 + qt * 128:q0 + (qt + 1) * 128],
                        tp[:, blk * 128:(blk + 1) * 128])
            for e in range(N_DT):
                if last:
                    zps = scp.tile([128, NQ], f32, tag="sc", name="zps")
                else:
                    zps = scr.tile([128, NQ], f32, tag="scr", name="zps")
                for c in range(2):
                    nc.tensor.matmul(
                        zps[:], wo_sb[:, c, e * 128:(e + 1) * 128],
                        ot[c][:, q0:q1], start=(c == 0), stop=(c == 1))
                zsb = zsbp.tile([128, NQ], fp16, tag="zsb", name="zsb")
                if last and e % 2 == 1:
                    nc.scalar.activation(zsb[:], zps[:], AF.Copy, scale=1.0)
                else:
                    nc.vector.tensor_copy(zsb[:], zps[:])
                nc.sync.dma_start(zT[e * 128:(e + 1) * 128, q0:q1], zsb[:])

    nc.compile()
    return nc


def _get_program():
    global _PROGRAM
    if _PROGRAM is None:
        _PROGRAM = _build_program()
    return _PROGRAM


ONESK_NP = None


def _init_consts():
    global ONESK_NP
    if ONESK_NP is None:
        ONESK_NP = np.ones((128, 1), np.float16)


def _make_in_maps(q, k, v, Wq, bq, Wk, Wv, Wo):
    _init_consts()
    f32 = np.float32
    xT = {}
    for b in range(B):
        xT[("q", b)] = np.ascontiguousarray(q[b].T, dtype=np.float16)
        xT[("k", b)] = np.ascontiguousarray(k[b].T, dtype=np.float16)
        xT[("v", b)] = np.ascontiguousarray(v[b].T, dtype=np.float16)
    wslices = {}
    for g in range(4):
        sl = slice(g * E, (g + 1) * E)
        wslices[("wq", g)] = np.ascontiguousarray(Wq[sl, :].T, dtype=np.float16)
        wslices[("wk", g)] = np.ascontiguousarray(Wk[sl, :].T, dtype=np.float16)
        wslices[("wv", g)] = np.ascontiguousarray(Wv[sl, :].T, dtype=np.float16)
        wslices[("wo", g)] = np.ascontiguousarray(Wo[:, sl].T, dtype=np.float16)
        wslices[("bq", g)] = np.ascontiguousarray(
            bq[sl].reshape(E, 1), dtype=f32)
    in_maps = []
    for c in range(N_CORES):
        b, g = c // 4, c % 4
        in_maps.append({
            "onesk": ONESK_NP,
            "qT": xT[("q", b)], "kT": xT[("k", b)], "vT": xT[("v", b)],
            "wq": wslices[("wq", g)], "wk": wslices[("wk", g)],
            "wv": wslices[("wv", g)], "wo": wslices[("wo", g)],
            "bq": wslices[("bq", g)],
        })
    return in_maps


def _numpy_fallback(q, k, v, mask, Wq, bq, Wk, bk, Wv, bv, Wo, bo):
    # Only used if mask is not all-True (never the case for this problem).
    def proj(x, W, b_):
        y = x @ W.T + b_
        return y.reshape(B, S, NUM_HEADS, DK).transpose(0, 2, 1, 3)
    qh, kh, vh = proj(q, Wq, bq), proj(k, Wk, bk), proj(v, Wv, bv)
    sc = np.einsum("bhqd,bhkd->bhqk", qh, kh) / np.sqrt(DK)
    sc = np.where(mask, sc, np.float32(-1e9))
    sc = sc - sc.max(-1, keepdims=True)
    p = np.exp(sc)
    p /= p.sum(-1, keepdims=True)
    o = np.einsum("bhqk,bhkd->bhqd", p, vh)
    o = o.transpose(0, 2, 1, 3).reshape(B, S, D_MODEL)
    return (o @ Wo.T + bo).astype(np.float32)


def kernel(q, k, v, mask, Wq, bq, Wk, bk, Wv, bv, Wo, bo):
    q = np.asarray(q, dtype=np.float32)
    k = np.asarray(k, dtype=np.float32)
    v = np.asarray(v, dtype=np.float32)
    Wq, Wk, Wv, Wo = (np.asarray(w, dtype=np.float32) for w in (Wq, Wk, Wv, Wo))
    bq, bk, bv, bo = (np.asarray(x, dtype=np.float32) for x in (bq, bk, bv, bo))
    if not np.all(np.asarray(mask)):
        return _numpy_fallback(q, k, v, np.asarray(mask), Wq, bq, Wk, bk,
                               Wv, bv, Wo, bo)

    from concourse.bass_utils import run_bass_kernel_spmd
    nc = _get_program()
    in_maps = _make_in_maps(q, k, v, Wq, bq, Wk, Wv, Wo)
    res = run_bass_kernel_spmd(nc, in_maps, core_ids=list(range(N_CORES)),
                               **_RUN_KWARGS)
    global _LAST_RESULTS
    _LAST_RESULTS = res
    # bk is dropped on-device (exact: softmax shift invariance); bv is
    # folded into the output bias (attention weights sum to 1).
    bo_eff = bo + Wo @ bv
    out = np.empty((B, S, D_MODEL), dtype=np.float32)
    for b in range(B):
        acc = res.results[4 * b]["zT"].astype(np.float32)
        for g in range(1, 4):
            acc = acc + res.results[4 * b + g]["zT"].astype(np.float32)
        out[b] = acc.T + bo_eff
    return out


# revision 16
# speedup vs baseline: 1.6171x; 1.0909x over previous
"""Trainium2 Bass kernel for nn_MultiHeadAttention (B=2, S=2048, d_model=1024, H=16).

Sharding (8 cores): data-parallel over B (2) x tensor-parallel over head groups
(4 groups of 4 heads).  Each core computes its head-group's Q/K/V projections
(column-sharded weights), attention for its 4 heads, and a row-parallel
out_proj partial product.  The host sums the 4 partials per batch (the
"all-reduce") and adds the output bias.

v3 design notes (cost-model driven):
  - P@V uses SWAPPED operands: P (exp scores, [k,q]) stationary, V ([k,e])
    moving, so the moving free dim is 64 instead of 512; attention output
    lands in [q, e].  Softmax denominators ride along as N=1 matmuls
    (rhs = ones) reusing the loaded P stationary tile.
  - Normalization = per-partition scalar multiply on DVE; PE transposes
    bring [q, e] back to [e, q] for the row-parallel out_proj.
  - PSUM is a single 8-bank working set shared by EVERYTHING (no stacked
    stage pools, which would serialize projections before attention):
    sc 2x2 banks, out2 2, sums 1, scratch 1.  Projections beyond the
    first k/q n-chunk are drip-fed through the scratch bank inside the
    attention loop (deadline-ordered backlog), so the ACT exp stream --
    the critical resource -- starts ~15us in instead of ~65us.
  - x is loaded in [128, d, 512] n-chunks (one DMA each) so the first
    chunk of K and Q arrives after ~9us of serial DMA instead of ~30us.
  - bk is dropped exactly (softmax shift invariance); bv is folded into
    the host-side output bias (attention weights sum to 1); bq is applied
    on-device during the qh PSUM->SBUF copy.
  - PSUM start_tensor_calc zeroing is bank-granular: only the first
    matmul touching a bank in an accumulation group sets start=True.
"""

import sys
import numpy as np

for _p in ("/opt/trn_rl_repo", "/root/.axon_site/_ro/trn_rl_repo"):
    if _p not in sys.path:
        sys.path.append(_p)

D_MODEL = 1024
NUM_HEADS = 16
DK = 64
B = 2
S = 2048
N_CORES = 8
HPC = 4               # heads per core
E = HPC * DK          # 256 features per core
NQ = 512              # q-chunk size
N_QC = S // NQ        # 4 q chunks
N_KT = S // 128       # 16 k tiles
N_DT = D_MODEL // 128  # 8 contraction tiles for projections

_PROGRAM = None
_RUN_KWARGS = {}      # test harness may set {"trace": True}
_LAST_RESULTS = None  # BassKernelResults of the last run

# Backlog draw schedule: how many deferred projection chunks to emit
# after each (qc, kt) iteration of the attention loop.
_DRAW = {0: [1, 1, 1, 1, 1, 1, 1, 1, 1, 2, 1, 2, 2, 2, 1, 1],
         1: [1, 1, 1, 1] + [0] * 12}


def _build_program():
    import concourse.bass as bass
    import concourse.mybir as mybir
    from concourse import bacc, tile
    from contextlib import ExitStack

    f32 = mybir.dt.float32
    fp16 = mybir.dt.float16
    i16 = mybir.dt.int16
    AF = mybir.ActivationFunctionType
    ALU = mybir.AluOpType
    # Schraudolph fast-exp constants (int16/fp16 bitcast):
    #   i16 = round(s * 0.125 * 1024/ln2 + (15*1024 - C));  C tuned for
    #   min RMS rel error (~1.8%); applied to ~22% of exp tiles on DVE.
    SCH_A = 0.125 * 1024.0 / np.log(2.0)
    SCH_B = 15.0 * 1024.0 - 60.0

    nc = bacc.Bacc("TRN2", target_bir_lowering=False, debug=False,
                   num_devices=N_CORES)

    fp8 = mybir.dt.float8e4
    DR = mybir.MatmulPerfMode.DoubleRow
    xdr = {}
    for nm in ("qTh", "qTl", "kTh", "kTl", "vTh", "vTl"):
        xdr[nm] = nc.dram_tensor(nm, [D_MODEL, S], fp8,
                                 kind="ExternalInput").ap()
    wdr = {}
    for nm in ("wqh", "wql", "wkh", "wkl", "wvh", "wvl"):
        wdr[nm] = nc.dram_tensor(nm, [D_MODEL, E], fp8,
                                 kind="ExternalInput").ap()
    wo = nc.dram_tensor("wo", [E, D_MODEL], fp16, kind="ExternalInput").ap()
    bq = nc.dram_tensor("bq", [E, 1], f32, kind="ExternalInput").ap()
    onesk = nc.dram_tensor("onesk", [128, 1], fp16, kind="ExternalInput").ap()
    zT = nc.dram_tensor("zT", [D_MODEL, S], fp16, kind="ExternalOutput").ap()

    with tile.TileContext(nc) as tc, ExitStack() as ctx:
        persist = ctx.enter_context(tc.tile_pool(name="persist", bufs=1))
        const = ctx.enter_context(tc.tile_pool(name="const", bufs=1))

        w_sb = {}
        for nm in ("wvh", "wvl", "wkh", "wkl", "wqh", "wql"):
            w_sb[nm] = persist.tile([128, N_DT, E], fp8, tag=nm, name=nm)
        wo_sb = persist.tile([128, 2, D_MODEL], fp16, tag="wo", name="wo")
        bq_sb = persist.tile([128, 2], f32, tag="bq", name="bq")

        from concourse.masks import make_identity
        ident = const.tile([128, 128], fp16, tag="ident", name="ident")
        make_identity(nc, ident)
        ones_k = const.tile([128, 1], fp16, tag="ones_k", name="ones_k")

        qh = [persist.tile([128, S], fp16, tag=f"qh{p}", name=f"qh{p}")
              for p in range(2)]
        kh = [persist.tile([128, S], fp16, tag=f"kh{p}", name=f"kh{p}")
              for p in range(2)]
        vh = persist.tile([128, N_KT, E], fp16, tag="vh", name="vh")
        ot = [persist.tile([128, S], fp16, tag=f"ot{p}", name=f"ot{p}")
              for p in range(2)]

        # ---- x chunk tiles + DMA schedule (priority order) --------------
        xpool = ctx.enter_context(tc.tile_pool(name="xpool", bufs=24))
        xt = {}
        for t in ("k", "q", "v"):
            for hl in "hl":
                xt[t + hl] = [xpool.tile([128, N_DT, NQ], fp8, tag="xt",
                                         name=f"x{t}{hl}{n}")
                              for n in range(4)]
        x3 = {k: v.rearrange("(t p) s -> p t s", p=128)
              for k, v in xdr.items()}

        def _xdma(eng, t, hl, n):
            eng.dma_start(xt[t + hl][n][:],
                          x3[t + "T" + hl][:, :, n * NQ:(n + 1) * NQ])

        def _wdma(eng, nm):
            eng.dma_start(w_sb[nm][:],
                          wdr[nm].rearrange("(t p) e -> p t e", p=128))

        # Everything on the SP queue: transfers serialize on the shared DMA
        # device regardless, and any DMA on the scalar queue would block the
        # ACT sequencer from issuing the (critical) exp stream.
        _wdma(nc.sync, "wvh")
        nc.sync.dma_start(ones_k[:], onesk)
        _wdma(nc.sync, "wvl")
        _xdma(nc.sync, "v", "h", 0)
        _xdma(nc.sync, "v", "l", 0)
        _wdma(nc.sync, "wkh")
        _wdma(nc.sync, "wkl")
        _xdma(nc.sync, "k", "h", 0)
        _xdma(nc.sync, "k", "l", 0)
        _wdma(nc.sync, "wqh")
        _wdma(nc.sync, "wql")
        _xdma(nc.sync, "q", "h", 0)
        _xdma(nc.sync, "q", "l", 0)
        nc.sync.dma_start(bq_sb[:], bq.rearrange("(m p) o -> p (m o)", p=128))
        for n in range(1, 4):
            _xdma(nc.sync, "v", "h", n)
            _xdma(nc.sync, "v", "l", n)
            _xdma(nc.sync, "k", "h", n)
            _xdma(nc.sync, "k", "l", n)
            _xdma(nc.sync, "q", "h", n)
            _xdma(nc.sync, "q", "l", n)
        nc.sync.dma_start(wo_sb[:], wo.rearrange("(t p) e -> p t e", p=128))

        # ---- PSUM pools: one shared 8-bank working set ------------------
        scp = ctx.enter_context(tc.tile_pool(name="scp", bufs=4, space="PSUM"))
        outp = ctx.enter_context(tc.tile_pool(name="outp", bufs=1, space="PSUM"))
        sump = ctx.enter_context(tc.tile_pool(name="sump", bufs=1, space="PSUM"))
        scr = ctx.enter_context(tc.tile_pool(name="scr", bufs=1, space="PSUM"))

        ptp = ctx.enter_context(tc.tile_pool(name="ptp", bufs=28))
        rp = ctx.enter_context(tc.tile_pool(name="rp", bufs=2))
        bcp = ctx.enter_context(tc.tile_pool(name="bcp", bufs=8))
        zsbp = ctx.enter_context(tc.tile_pool(name="zsbp", bufs=4))

        # ---- first K/Q n-chunk on the (still idle) score slots ----------
        # weights are host-scaled by 32 (fp8e4 subnormal avoidance); the
        # PSUM->SBUF copy applies the 1/32.  3 passes: wh@xh + wh@xl + wl@xh.
        def _dr_passes(t, wn):
            return ((w_sb[wn + "h"], xt[t + "h"]),
                    (w_sb[wn + "h"], xt[t + "l"]),
                    (w_sb[wn + "l"], xt[t + "h"]))

        def proj_big(t, wn, dst, n, bias):
            for m in range(2):
                ps = scp.tile([128, NQ], f32, tag="sc", name="projbig")
                passes = _dr_passes(t, wn)
                for pi, (wsb, xs) in enumerate(passes):
                    for dp in range(N_DT // 2):
                        nc.tensor.matmul(
                            ps[:], wsb[:, 2 * dp:2 * dp + 2,
                                       m * 128:(m + 1) * 128],
                            xs[n][:, 2 * dp:2 * dp + 2, :],
                            start=(pi == 0 and dp == 0),
                            stop=(pi == 2 and dp == N_DT // 2 - 1),
                            perf_mode=DR)
                if bias is None:
                    nc.vector.tensor_scalar_mul(
                        dst[m][:, n * NQ:(n + 1) * NQ], ps[:], 1.0 / 32)
                else:
                    nc.vector.tensor_scalar(
                        dst[m][:, n * NQ:(n + 1) * NQ], ps[:], 1.0 / 32,
                        bias[:, m:m + 1], ALU.mult, ALU.add)

        # V0-3 run during the kT/qT DMA wait and warm up the PE p-state
        # (they only need wv + the first vT chunk, which load first).
        # Dummy identity transposes (never read) fill the remaining DMA-wait
        # gaps so the p-state ramp reaches full speed before Kn0/Qn0.
        _V_PRE = 4
        wtp = scp.tile([128, 1024], fp16, tag="sc", name="wtp")

        def warm(cnt):
            for i in range(cnt):
                nc.tensor.matmul(
                    wtp[:, (i % 8) * 128:(i % 8 + 1) * 128], ident[:],
                    ident[:], is_transpose=True, start=True, stop=True,
                    skip_group_check=True)

        # ---- deferred projection backlog (drip-fed through scratch) -----
        def emit_v(st):
            vps = scr.tile([128, E], f32, tag="scr", name="vps")
            n, col = st // 4, (st % 4) * 128
            passes = ((xt["vh"], w_sb["wvh"]), (xt["vl"], w_sb["wvh"]),
                      (xt["vh"], w_sb["wvl"]))
            for pi, (xs, wsb) in enumerate(passes):
                for dp in range(N_DT // 2):
                    nc.tensor.matmul(
                        vps[:], xs[n][:, 2 * dp:2 * dp + 2, col:col + 128],
                        wsb[:, 2 * dp:2 * dp + 2, :],
                        start=(pi == 0 and dp == 0),
                        stop=(pi == 2 and dp == N_DT // 2 - 1),
                        perf_mode=DR)
            nc.vector.tensor_scalar_mul(vh[:, st, :], vps[:], 1.0 / 32)

        def emit_kq_chunk(t, wn, dst, n, m, bias):
            ps = scr.tile([128, NQ], f32, tag="scr", name="kqps")
            passes = _dr_passes(t, wn)
            for pi, (wsb, xs) in enumerate(passes):
                for dp in range(N_DT // 2):
                    nc.tensor.matmul(
                        ps[:], wsb[:, 2 * dp:2 * dp + 2,
                                   m * 128:(m + 1) * 128],
                        xs[n][:, 2 * dp:2 * dp + 2, :],
                        start=(pi == 0 and dp == 0),
                        stop=(pi == 2 and dp == N_DT // 2 - 1),
                        perf_mode=DR)
            if bias is None:
                nc.vector.tensor_scalar_mul(
                    dst[m][:, n * NQ:(n + 1) * NQ], ps[:], 1.0 / 32)
            else:
                nc.vector.tensor_scalar(
                    dst[m][:, n * NQ:(n + 1) * NQ], ps[:], 1.0 / 32,
                    bias[:, m:m + 1], ALU.mult, ALU.add)

        warm(28)
        for st in range(_V_PRE):
            emit_v(st)
            warm(8)
        proj_big("k", "wk", kh, 0, None)
        warm(6)
        proj_big("q", "wq", qh, 0, bq_sb)

        backlog = []
        _K = lambda n, m: (lambda: emit_kq_chunk("k", "wk", kh, n, m, None))
        _Q = lambda n, m: (lambda: emit_kq_chunk("q", "wq", qh, n, m, bq_sb))
        _V = lambda st: (lambda: emit_v(st))
        backlog += [_V(4), _K(1, 0), _V(5), _K(1, 1), _V(6), _K(2, 0),
                    _V(7), _K(2, 1), _V(8), _V(9), _K(3, 0), _V(10),
                    _V(11), _K(3, 1), _V(12), _Q(1, 0), _V(13), _Q(1, 1),
                    _V(14), _V(15), _Q(2, 0), _Q(2, 1), _Q(3, 0), _Q(3, 1)]
        backlog = backlog[::-1]  # pop() from the front

        # ---- attention + out_proj, per q-chunk --------------------------
        for qc in range(N_QC):
            q0, q1 = qc * NQ, (qc + 1) * NQ
            out2 = outp.tile([128, 4, E], f32, tag="out2", name="out2")
            sums = sump.tile([128, 16], f32, tag="sums", name="sums")

            def pv_sums(kt, pts):
                # only the FIRST matmul touching each PSUM bank of an
                # accumulation group may set start=True (bank-granular zero)
                for h in range(4):
                    for qt in range(4):
                        lhsT = pts[h][:, qt * 128:(qt + 1) * 128]
                        nc.tensor.matmul(
                            out2[:, qt, h * 64:(h + 1) * 64], lhsT,
                            vh[:, kt, h * 64:(h + 1) * 64],
                            start=(kt == 0 and h == 0 and qt % 2 == 0),
                            stop=(kt == N_KT - 1),
                            skip_group_check=True)
                        nc.tensor.matmul(
                            sums[:, qt * 4 + h:qt * 4 + h + 1], lhsT,
                            ones_k[:],
                            start=(kt == 0 and h == 0 and qt == 0),
                            stop=(kt == N_KT - 1),
                            skip_group_check=True)

            draw = _DRAW.get(qc, [0] * N_KT)
            prev_pts = None
            for kt in range(N_KT):
                k0 = kt * 128
                scs = []
                for h in range(4):
                    p, j = h // 2, h % 2
                    lo, hi = j * 64, (j + 1) * 64
                    sc = scp.tile([128, NQ], f32, tag="sc", name="sc")
                    nc.tensor.matmul(
                        sc[:], kh[p][lo:hi, k0:k0 + 128],
                        qh[p][lo:hi, q0:q1], start=True, stop=True)
                    scs.append(sc)
                if prev_pts is not None:
                    pv_sums(kt - 1, prev_pts)
                for _ in range(draw[kt]):
                    if backlog:
                        backlog.pop()()
                pts = []
                for h in range(4):
                    off = ((kt * 4 + h) % 16 == 9 if qc == 0
                           else (kt * 4 + h) % 8 in (1, 4, 6))
                    if off:
                        pti = ptp.tile([128, NQ], i16, tag="pt", name="pti")
                        nc.vector.tensor_scalar(
                            pti[:], scs[h][:], SCH_A, SCH_B,
                            ALU.mult, ALU.add)
                        pt = pti.bitcast(fp16)
                    else:
                        pt = ptp.tile([128, NQ], fp16, tag="pt", name="pt")
                        nc.scalar.activation(pt[:], scs[h][:], AF.Exp,
                                             scale=0.125)
                    pts.append(pt)
                prev_pts = pts
            pv_sums(N_KT - 1, prev_pts)

            # drain: normalize in [q, e], transpose to [e, q], out_proj.
            # All 16 normalize blocks go first (DVE/ACT alternating on the
            # last chunk), then all transposes, then all ot copies, so the
            # engines pipeline instead of chaining.  The out_proj of qc<3 is
            # deferred into the NEXT q-chunk's backlog (so it never blocks
            # the loop); qc3's runs on the freed score slots.
            last = qc == N_QC - 1
            rv = rp.tile([128, 16], f32, tag="rv", name="rv")
            nc.vector.reciprocal(rv[:], sums[:])
            o2n = []
            for qt in range(4):
                o2 = bcp.tile([128, E], fp16, tag="o2n", name="o2n")
                o2n.append(o2)
            for qt in range(4):
                for h in range(4):
                    c0 = qt * 4 + h
                    if last and h % 2 == 1:
                        nc.scalar.activation(
                            o2n[qt][:, h * 64:(h + 1) * 64],
                            out2[:, qt, h * 64:(h + 1) * 64],
                            AF.Copy, scale=rv[:, c0:c0 + 1])
                    else:
                        nc.vector.tensor_scalar_mul(
                            o2n[qt][:, h * 64:(h + 1) * 64],
                            out2[:, qt, h * 64:(h + 1) * 64],
                            rv[:, c0:c0 + 1])
            tp = scr.tile([128, 1024], fp16, tag="scr", name="tp")
            for qt in range(4):
                for et in range(2):
                    blk = qt * 2 + et
                    nc.tensor.matmul(
                        tp[:, blk * 128:(blk + 1) * 128],
                        o2n[qt][:, et * 128:(et + 1) * 128], ident[:],
                        is_transpose=True, start=True, stop=True,
                        skip_group_check=True)
            for qt in range(4):
                for et in range(2):
                    blk = qt * 2 + et
                    if last and blk % 2 == 1:
                        nc.scalar.activation(
                            ot[et][:, q0 + qt * 128:q0 + (qt + 1) * 128],
                            tp[:, blk * 128:(blk + 1) * 128],
                            AF.Copy, scale=1.0)
                    else:
                        nc.vector.tensor_copy(
                            ot[et][:, q0 + qt * 128:q0 + (qt + 1) * 128],
                            tp[:, blk * 128:(blk + 1) * 128])

            def out_proj_chunk(qc_, e, on_sc):
                q0_, q1_ = qc_ * NQ, (qc_ + 1) * NQ
                if on_sc:
                    zps = scp.tile([128, NQ], f32, tag="sc", name="zps")
                else:
                    zps = scr.tile([128, NQ], f32, tag="scr", name="zps")
                for c in range(2):
                    nc.tensor.matmul(
                        zps[:], wo_sb[:, c, e * 128:(e + 1) * 128],
                        ot[c][:, q0_:q1_], start=(c == 0), stop=(c == 1))
                zsb = zsbp.tile([128, NQ], fp16, tag="zsb", name="zsb")
                if on_sc and e % 2 == 1:
                    nc.scalar.activation(zsb[:], zps[:], AF.Copy, scale=1.0)
                else:
                    nc.vector.tensor_copy(zsb[:], zps[:])
                nc.sync.dma_start(zT[e * 128:(e + 1) * 128, q0_:q1_], zsb[:])

            if last:
                for e in range(N_DT):
                    out_proj_chunk(qc, e, True)
            else:
                for e in range(N_DT):
                    out_proj_chunk(qc, e, False)

    nc.compile()
    return nc


def _get_program():
    global _PROGRAM
    if _PROGRAM is None:
        _PROGRAM = _build_program()
    return _PROGRAM


ONESK_NP = None


def _init_consts():
    global ONESK_NP
    if ONESK_NP is None:
        ONESK_NP = np.ones((128, 1), np.float16)


def _hilo(a, f8):
    hi = a.astype(f8)
    lo = (a - hi.astype(np.float32)).astype(f8)
    return np.ascontiguousarray(hi), np.ascontiguousarray(lo)


def _make_in_maps(q, k, v, Wq, bq, Wk, Wv, Wo):
    _init_consts()
    import ml_dtypes
    f8 = ml_dtypes.float8_e4m3
    f32 = np.float32
    xT = {}
    for b in range(B):
        for nm, x in (("q", q), ("k", k), ("v", v)):
            h, lo = _hilo(np.ascontiguousarray(x[b].T), f8)
            xT[(nm, b)] = (h, lo)
    wslices = {}
    for g in range(4):
        sl = slice(g * E, (g + 1) * E)
        for nm, W in (("wq", Wq), ("wk", Wk), ("wv", Wv)):
            h, lo = _hilo(np.ascontiguousarray(W[sl, :].T) * 32.0, f8)
            wslices[(nm, g)] = (h, lo)
        wslices[("wo", g)] = np.ascontiguousarray(Wo[:, sl].T, dtype=np.float16)
        wslices[("bq", g)] = np.ascontiguousarray(
            bq[sl].reshape(E, 1), dtype=f32)
    in_maps = []
    for c in range(N_CORES):
        b, g = c // 4, c % 4
        in_maps.append({
            "onesk": ONESK_NP,
            "qTh": xT[("q", b)][0], "qTl": xT[("q", b)][1],
            "kTh": xT[("k", b)][0], "kTl": xT[("k", b)][1],
            "vTh": xT[("v", b)][0], "vTl": xT[("v", b)][1],
            "wqh": wslices[("wq", g)][0], "wql": wslices[("wq", g)][1],
            "wkh": wslices[("wk", g)][0], "wkl": wslices[("wk", g)][1],
            "wvh": wslices[("wv", g)][0], "wvl": wslices[("wv", g)][1],
            "wo": wslices[("wo", g)], "bq": wslices[("bq", g)],
        })
    return in_maps


def _numpy_fallback(q, k, v, mask, Wq, bq, Wk, bk, Wv, bv, Wo, bo):
    # Only used if mask is not all-True (never the case for this problem).
    def proj(x, W, b_):
        y = x @ W.T + b_
        return y.reshape(B, S, NUM_HEADS, DK).transpose(0, 2, 1, 3)
    qh, kh, vh = proj(q, Wq, bq), proj(k, Wk, bk), proj(v, Wv, bv)
    sc = np.einsum("bhqd,bhkd->bhqk", qh, kh) / np.sqrt(DK)
    sc = np.where(mask, sc, np.float32(-1e9))
    sc = sc - sc.max(-1, keepdims=True)
    p = np.exp(sc)
    p /= p.sum(-1, keepdims=True)
    o = np.einsum("bhqk,bhkd->bhqd", p, vh)
    o = o.transpose(0, 2, 1, 3).reshape(B, S, D_MODEL)
    return (o @ Wo.T + bo).astype(np.float32)


def kernel(q, k, v, mask, Wq, bq, Wk, bk, Wv, bv, Wo, bo):
    q = np.asarray(q, dtype=np.float32)
    k = np.asarray(k, dtype=np.float32)
    v = np.asarray(v, dtype=np.float32)
    Wq, Wk, Wv, Wo = (np.asarray(w, dtype=np.float32) for w in (Wq, Wk, Wv, Wo))
    bq, bk, bv, bo = (np.asarray(x, dtype=np.float32) for x in (bq, bk, bv, bo))
    if not np.all(np.asarray(mask)):
        return _numpy_fallback(q, k, v, np.asarray(mask), Wq, bq, Wk, bk,
                               Wv, bv, Wo, bo)

    from concourse.bass_utils import run_bass_kernel_spmd
    nc = _get_program()
    in_maps = _make_in_maps(q, k, v, Wq, bq, Wk, Wv, Wo)
    res = run_bass_kernel_spmd(nc, in_maps, core_ids=list(range(N_CORES)),
                               **_RUN_KWARGS)
    global _LAST_RESULTS
    _LAST_RESULTS = res
    # bk is dropped on-device (exact: softmax shift invariance); bv is
    # folded into the output bias (attention weights sum to 1).
    bo_eff = bo + Wo @ bv
    out = np.empty((B, S, D_MODEL), dtype=np.float32)
    for b in range(B):
        acc = res.results[4 * b]["zT"].astype(np.float32)
        for g in range(1, 4):
            acc = acc + res.results[4 * b + g]["zT"].astype(np.float32)
        out[b] = acc.T + bo_eff
    return out
